# revision 15
# baseline (speedup 1.0000x reference)
"""Trainium2 Bass kernel for nn_DE_NN_35820027249305 (dense_mlp, memory regime).

Reference computation (per particle l, per batch element b, x = X[l,0,b]):
    y = w4 @ relu(W3 @ relu(W2 @ relu(w1 * x)))
The MLP has no biases, so each particle's scalar->scalar map is positively
homogeneous: f(x) = x*f(1) for x>=0 and f(x) = -x*f(-1) for x<0.  The weights
therefore fold (on host, 44*72 flops) into two per-particle slopes a = f(1),
b = -f(-1), and the kernel becomes the purely memory-bound elementwise stream
    y = a*x + (b-a)*min(x, 0)

Device kernel per core (batch-sharded, 400000/8 = 50000 per core, all 44
particles): data laid out as (880, 2500) so every SBUF partition row maps to
exactly one particle; per tile [128, 2500]:
    ACT:  u = |x|
    DVE:  u = u * c2[p]          (tensor_scalar, 2x mode)
    DVE:  y = (x * c1[p]) + u    (scalar_tensor_tensor)
DMA in/out ~17.9 MB per core => ~50 us HBM roofline.
"""

from contextlib import ExitStack

import numpy as np

import concourse.bass as bass
import concourse.mybir as mybir
from concourse.bass_utils import run_bass_kernel_spmd

# Problem constants (hardcoded per the harness contract).
N_PART = 44          # particles
BATCH = 400000       # full batch
N_CORES = 8
B_CORE = BATCH // N_CORES      # 50000 batch elements per core
F = 2500                       # free-dim tile width
RPP = B_CORE // F              # rows per particle = 20
ROWS = N_PART * RPP            # 880 rows per core
P = 128
NT = (ROWS + P - 1) // P       # 7 tiles (last has 112 rows)
NBUF = 4                       # buffer slots per stream (x / u / y)

_CACHED = {}


def _build_kernel():
    """Raw-bass kernel with explicit semaphores.

    The walrus build in this container allows at most ONE semaphore wait
    embedded per instruction, so Tile's auto-generated multi-wait sync does
    not compile.  Raw bass lets us issue standalone wait_ge instructions
    (EventSemaphore ops, one wait each) and keep every DMA/compute
    instruction wait-free.

    Engine programs:
      SP  (nc.sync):   coefficient DMA + x-tile loads      (qSPDynamicHW)
      ACT (nc.scalar): y-tile stores                        (qActDynamicHW)
      DVE (nc.vector): per tile
            u = min(x, 0) * c2          -- TensorScalar (min, mult)
            y = (x * c1) + u            -- scalar_tensor_tensor (mult, add)
    """
    if "nc" in _CACHED:
        return _CACHED["nc"]
    f32 = mybir.dt.float32
    nc = bass.Bass()
    x_in = nc.declare_dram_parameter("x_in", [ROWS, F], f32, isOutput=False)
    cm = nc.declare_dram_parameter("cm", [P, 2 * NT], f32, isOutput=False)
    y_out = nc.declare_dram_parameter("y_out", [ROWS, F], f32, isOutput=True)

    ctx = ExitStack()
    with ctx:
        cms = ctx.enter_context(nc.sbuf_tensor("cms", [P, 2 * NT], f32))
        xb = [
            ctx.enter_context(nc.sbuf_tensor(f"xb{i}", [P, F], f32))
            for i in range(NBUF)
        ]
        ub = [
            ctx.enter_context(nc.sbuf_tensor(f"ub{i}", [P, F], f32))
            for i in range(NBUF)
        ]
        yb = [
            ctx.enter_context(nc.sbuf_tensor(f"yb{i}", [P, F], f32))
            for i in range(NBUF)
        ]
        s_cm = ctx.enter_context(nc.semaphore("s_cm"))
        s_load = ctx.enter_context(nc.semaphore("s_load"))
        s_comp = ctx.enter_context(nc.semaphore("s_comp"))
        s_store = ctx.enter_context(nc.semaphore("s_store"))
        block = ctx.enter_context(nc.Block())

        @block.sync
        def _(sync):
            sync.dma_start(cms[:], cm[:]).then_inc(s_cm, 16)
            for t in range(NT):
                if t >= NBUF:
                    # xb/ub slot free once compute of tile t-NBUF finished
                    sync.wait_ge(s_comp, t - NBUF + 1)
                r0 = t * P
                p = min(P, ROWS - r0)
                sync.dma_start(xb[t % NBUF][:p], x_in[r0 : r0 + p, :]).then_inc(
                    s_load, 16
                )

        @block.scalar
        def _(scalar):
            for t in range(NT):
                r0 = t * P
                p = min(P, ROWS - r0)
                scalar.wait_ge(s_comp, t + 1)  # y tile t ready
                scalar.dma_start(y_out[r0 : r0 + p, :], yb[t % NBUF][:p]).then_inc(
                    s_store, 16
                )
            scalar.wait_ge(s_store, 16 * NT)  # all outputs landed in HBM

        @block.vector
        def _(vector):
            vector.wait_ge(s_cm, 16)
            for t in range(NT):
                i = t % NBUF
                p = min(P, ROWS - t * P)
                vector.wait_ge(s_load, 16 * (t + 1))  # x tile t in SBUF
                if t >= NBUF:
                    # yb slot drained by store of tile t-NBUF
                    vector.wait_ge(s_store, 16 * (t - NBUF + 1))
                # u = min(x, 0) * c2   with c2 = b - a
                vector.tensor_scalar(
                    ub[i][:p],
                    xb[i][:p],
                    0.0,
                    cms[:p, NT + t : NT + t + 1],
                    mybir.AluOpType.min,
                    mybir.AluOpType.mult,
                )
                vector.scalar_tensor_tensor(
                    yb[i][:p],
                    xb[i][:p],
                    cms[:p, t : t + 1],
                    ub[i][:p],
                    mybir.AluOpType.mult,
                    mybir.AluOpType.add,
                ).then_inc(s_comp, 1)

    _CACHED["nc"] = nc
    return nc


def _fold_weights(lin1s, lin2s, lin3s, lin4s):
    """Collapse each particle's bias-free ReLU MLP into slopes (a, b):
    f(x) = a*x for x>0, b*x for x<0.  Returns c1 = a, c2 = b - a."""

    def f(xval):
        x = np.full((N_PART, 1, 1), xval, dtype=np.float32)
        h = np.maximum(np.einsum("lik,lkj->lij", lin1s, x), 0.0).astype(np.float32)
        h = np.maximum(np.einsum("lik,lkj->lij", lin2s, h), 0.0).astype(np.float32)
        h = np.maximum(np.einsum("lik,lkj->lij", lin3s, h), 0.0).astype(np.float32)
        return np.einsum("lik,lkj->lij", lin4s, h)[:, 0, 0].astype(np.float32)

    a = f(1.0)
    b = -f(-1.0)
    # y = c1*x + c2*min(x, 0)  with c1 = a, c2 = b - a
    c1 = a.astype(np.float32)
    c2 = (b - a).astype(np.float32)
    return c1, c2


def _make_in_maps(X, lin1s, lin2s, lin3s, lin4s):
    X = np.asarray(X, dtype=np.float32)
    c1, c2 = _fold_weights(
        np.asarray(lin1s, dtype=np.float32),
        np.asarray(lin2s, dtype=np.float32),
        np.asarray(lin3s, dtype=np.float32),
        np.asarray(lin4s, dtype=np.float32),
    )

    # Per-partition-row coefficient maps: row r of the (ROWS, F) layout holds
    # data of particle r // RPP.  Same for every core (batch sharding).
    row_particle = np.arange(NT * P) // RPP          # len 896; rows >= 880 pad
    row_particle = np.minimum(row_particle, N_PART - 1)
    c1_map = c1[row_particle].reshape(NT, P).T  # [P, NT]
    c2_map = c2[row_particle].reshape(NT, P).T
    cm_map = np.ascontiguousarray(
        np.concatenate([c1_map, c2_map], axis=1), dtype=np.float32
    )  # [P, 2*NT]

    in_maps = []
    for c in range(N_CORES):
        shard = np.ascontiguousarray(
            X[:, 0, c * B_CORE : (c + 1) * B_CORE]
        ).reshape(ROWS, F)
        in_maps.append({"x_in": shard, "cm": cm_map})
    return in_maps


def _gather(results):
    out = np.empty((N_PART, 1, BATCH), dtype=np.float32)
    for c in range(N_CORES):
        y = results[c]["y_out"].reshape(N_PART, B_CORE)
        out[:, 0, c * B_CORE : (c + 1) * B_CORE] = y
    return out


def kernel(X, lin1s, lin2s, lin3s, lin4s):
    nc = _build_kernel()
    in_maps = _make_in_maps(X, lin1s, lin2s, lin3s, lin4s)
    res = run_bass_kernel_spmd(nc, in_maps, core_ids=list(range(N_CORES)))
    return _gather(res.results)
